# revision 12
# baseline (speedup 1.0000x reference)
"""Trainium2 Bass kernel for nn_MultiHeadAttention_5360119185803.

Full-d_model attention (no head split) + residual + LayerNorm, B=4, T=S=2048,
E=1024, fp32 in/out.

Sharding: 8 cores; core c owns batch b=c//2 and query rows
[(c%2)*1024, (c%2+1)*1024). K/V is full per batch; the core pair duplicates
the (tiny) K/V-side work (collectives measured slower than recompute).

v8 design (fp32r 462us -> fp8 DR v5 236us -> v6 122us -> v7 121us -> this).
The PE is at the fp8 DR roofline (~86us of matmul per core); v8 removes the
remaining non-PE time found in the v7 trace:
  * Input DMA is HBM bound (~280 GB/s aggregate, ~140/queue): the critical
    xq+wqk (2MB) lands ~15.5us in, and xk (2MB) behind it gated P4.  v8
    interleaves streams across both queues so each jp chunk-pair (wqk pair +
    xq pair) lands every ~1.8us, and splits xk across both queues right
    after (lands ~21.5us, before P4 needs it).
  * P3 restructured to consume chunk-pairs AS THEY ARRIVE: pass 1 (t-half 0)
    runs jp-major with 8 concurrent psum banks — each jp batch (8 matmuls,
    1.73us) matches the pair arrival cadence, with a couple of zero-junk
    accumulate matmuls (adds 0.0 to open psums) after each batch as p-state
    insurance; pass 2 (t-half 1) runs et-major with everything resident,
    overlapping pass 1's evicts.  P3 ends ~24us vs v7's ~28.5.
  * P4 runs t-half 0 for all 16 s-tiles, then t-half 1, so it only ever
    waits on already-evicted qk8 columns.
  * Tail: the two tiles of each quarter interleave their P6 matmuls per
    gc-pair so both tiles' early-quarter y/stats hide under matmuls; y-STT
    alternates DVE (even quarters) / GpSimd (odd) to halve the exposed DVE
    chain; final normalize h0 on ACT || h1 on GpSimd (tensor_scalar); out
    stores h0 on the sync queue || h1 on the scalar queue.
Carried from v7: single shared PSUM pool (4 tags x bufs=2 = 8 banks, later
phases rotate into earlier tags; rotation deps land on long-done evicts),
PE-transpose of the rowsum (no DRAM roundtrip), residual prefetched in bf16.
Carried from v6 (math): Wqk = Wq.T@Wk, Wvo = Wv.T@Wo.T, bo' = bo + Wo@bv,
bk dropped; all GEMMs fp8e4 DoubleRow (K=256/pass, 1 col/cyc = 157 TF/s);
weights stored as 32*W; exp(psum/1024 - 2) evict; recip = 1/(16*rowsum).

kernel() is self-contained: host prep = shard + dtype converts + weight folds.
"""

import sys

sys.path.insert(0, "/opt/trn_rl_repo")

import ml_dtypes
import numpy as np

import concourse.bacc as bacc
import concourse.bass as bass
import concourse.tile as tile
from concourse import mybir
from concourse.bass_utils import run_bass_kernel_spmd

P = 128
E = 1024          # d_model
S = 2048          # kv seq len per batch
T = 1024          # query rows per core
NE = E // P       # 8 chunks of contraction dim
NT = T // P       # 8 t tiles
NS = S // P       # 16 s tiles
FD = 512          # matmul moving free dim / PSUM bank
QD = 256          # quarter width in t/g columns
NBLK_T = T // FD  # 2 blocks of 512
NP = NE // 2      # 4 DoubleRow pair-chunks over e/f
NSP = NS // 2     # 8 DoubleRow pair-chunks over s
NWUP = 24         # warmup junk matmuls before P3
NJF = 3           # zero-junk accumulates after each jp batch in P3 pass 1

f32 = mybir.dt.float32
bf16 = mybir.dt.bfloat16
f8 = mybir.dt.float8e4
AF = mybir.ActivationFunctionType
ALU = mybir.AluOpType
DR = mybir.MatmulPerfMode.DoubleRow

_cache = {}


def _build(apply_gb):
    nc = bacc.Bacc("TRN2", target_bir_lowering=False, debug=False, num_devices=8)

    xqT8d = nc.dram_tensor("xqT8", [E, T], f8, kind="ExternalInput")
    xkT8d = nc.dram_tensor("xkT8", [E, S], f8, kind="ExternalInput")
    xv8d = nc.dram_tensor("xv8", [S, E], f8, kind="ExternalInput")
    resd = nc.dram_tensor("resb", [T, E], bf16, kind="ExternalInput")  # xq+bo'
    wqk8 = nc.dram_tensor("wqk8", [E, E], f8, kind="ExternalInput")  # 32*Wq.T@Wk
    wvo8 = nc.dram_tensor("wvo8", [E, E], f8, kind="ExternalInput")  # 32*Wv.T@Wo.T
    ck2 = nc.dram_tensor("ck2", [P, NE], f32, kind="ExternalInput")  # 32*Wk.T@bq
    if apply_gb:
        gam = nc.dram_tensor("gam", [E], f32, kind="ExternalInput")
        bet = nc.dram_tensor("bet", [E], f32, kind="ExternalInput")
    out = nc.dram_tensor("out", [T, E], f32, kind="ExternalOutput")

    with tile.TileContext(nc) as tc:
        consts = tc.alloc_tile_pool(name="consts", bufs=1, side="left")
        junk8 = consts.tile([P, 2, P], f8)
        nc.gpsimd.memset(junk8, 0.0)  # first: warmup depends only on this
        jmov = consts.tile([P, 2, FD], f8)
        nc.vector.memset(jmov, 0.0)  # DVE is idle early; needed only ~12us

        # ---- PSUM: one pool, 4 tags x bufs=2 = all 8 banks.  Every phase
        # rotates through tags b0..b3; rotation deps land on long-completed
        # evicts, so no phase-boundary stalls ----
        mmp = tc.alloc_tile_pool(name="mmp", bufs=2, space="PSUM")
        jfill = mmp.tile([P, FD], f32, name="jfill", tag="b0")

        def junk_mm():
            nc.tensor.matmul(jfill[:, 0:P], junk8, junk8, start=True,
                             stop=True, perf_mode=DR)

        # PE warmup burst (HAM ramp) — no input deps
        for _ in range(NWUP):
            junk_mm()

        # ---- input DMA, interleaved across the two queues so each jp
        # chunk-pair (wqk + xq) completes every ~1.8us, then xk split ----
        wpool = tc.alloc_tile_pool(name="wpool", bufs=1, side="left")
        wqk_sb = wpool.tile([P, NE, E], f8)
        wqk_r = wqk8.ap().rearrange("(j p) f -> j p f", p=P)
        xqT_pool = tc.alloc_tile_pool(name="xqT", bufs=1, side="left")
        xqT8 = xqT_pool.tile([P, NE, T], f8)
        xq_r = xqT8d.ap().rearrange("(j p) t -> j p t", p=P)
        xkT_pool = tc.alloc_tile_pool(name="xkT", bufs=1, side="left")
        xkT8 = xkT_pool.tile([P, NE, S], f8)
        xk_r = xkT8d.ap().rearrange("(j p) s -> j p s", p=P)

        # pair k = (wqk c2k,c2k+1 + xq c2k,c2k+1) feeds P3 pass-1's jp=k
        # batch.  Chunks round-robin over sync/scalar/gpsimd so pair k
        # completes ~10.8 + 1.5k us; priority gates (tiny reads of the
        # critical tiles) keep each queue's later streams from starving the
        # in-flight critical slices (HBM arbitration is not fair).
        PAIR_Q = [  # (tensor, chunk, queue) in per-queue trigger order
            (0, "wqk", "sync"), (1, "wqk", "scalar"), (0, "xq", "gpsimd"),
            (1, "xq", "sync"),
            (2, "wqk", "scalar"), (3, "wqk", "gpsimd"), (2, "xq", "sync"),
            (3, "xq", "scalar"),
            (5, "wqk", "gpsimd"), (4, "wqk", "sync"), (4, "xq", "scalar"),
            (5, "xq", "sync"),
            (6, "wqk", "gpsimd"), (7, "wqk", "scalar"), (7, "xq", "gpsimd"),
            (6, "xq", "sync"),
        ]
        for j, which, q in PAIR_Q:
            eng = {"sync": nc.sync, "scalar": nc.scalar,
                   "gpsimd": nc.gpsimd}[q]
            if which == "wqk":
                eng.dma_start(out=wqk_sb[:, j, :], in_=wqk_r[j])
            else:
                eng.dma_start(out=xqT8[:, j, :], in_=xq_r[j])
        ck_sb = consts.tile([P, NE], f32)
        nc.scalar.dma_start(out=ck_sb, in_=ck2.ap())
        # gates: xk waits for all critical chunks; res/xv wait for xk
        gdr = nc.dram_tensor("gate_scratch", [4, NE, 1], f8)
        gts = consts.tile([1, NE, 1], f8)
        nc.sync.dma_start(out=gdr.ap()[0], in_=xqT8[0:1, :, 0:1])
        nc.sync.dma_start(out=gdr.ap()[1], in_=wqk_sb[0:1, :, 0:1])
        nc.gpsimd.tensor_copy(gts, xqT8[0:1, :, 0:1])
        nc.gpsimd.tensor_copy(gts, wqk_sb[0:1, :, 0:1])
        for j in range(4):
            nc.gpsimd.dma_start(out=xkT8[:, j + 4, :], in_=xk_r[j + 4])
            nc.sync.dma_start(out=xkT8[:, j, :], in_=xk_r[j])
        nc.sync.dma_start(out=gdr.ap()[2], in_=xkT8[0:1, :, 0:1])
        nc.gpsimd.tensor_copy(gts, xkT8[0:1, :, 0:1])
        # small consts (needed from P4 on)
        eps_t = consts.tile([P, 1], f32)
        nc.gpsimd.memset(eps_t, 1e-6)
        neg2_t = consts.tile([P, 1], f32)
        nc.gpsimd.memset(neg2_t, -2.0)
        ones8 = consts.tile([P, 2, 16], f8)
        nc.gpsimd.memset(ones8, 1.0)
        id1 = consts.tile([1, 1], f32)
        nc.gpsimd.memset(id1, 1.0)
        recip_t = consts.tile([P, NT], f32)
        # res prefetch (sync) and xv/wvo (gpsimd): needed only from ~52us on
        res_pool = tc.alloc_tile_pool(name="resp", bufs=1, side="right")
        res_t = res_pool.tile([P, NT, E], bf16)
        res_r = resd.ap().rearrange("(tt p) e -> tt p e", p=P)
        for tt in range(NT):
            nc.sync.dma_start(out=res_t[:, tt, :], in_=res_r[tt])
        v_pool = tc.alloc_tile_pool(name="v8", bufs=1, side="left")
        v8 = v_pool.tile([P, NS, E], f8)
        xv_r = xv8d.ap().rearrange("(st p) e -> st p e", p=P)
        for st in range(NS):
            nc.gpsimd.dma_start(out=v8[:, st, :], in_=xv_r[st])
        wvo_sb = wpool.tile([P, NE, E], f8)
        wvo_r = wvo8.ap().rearrange("(j p) f -> j p f", p=P)
        for j in range(NE):
            nc.gpsimd.dma_start(out=wvo_sb[:, j, :], in_=wvo_r[j])
        if apply_gb:
            gam_sb = consts.tile([P, E], f32)
            nc.gpsimd.dma_start(out=gam_sb, in_=gam.ap().partition_broadcast(P))
            bet_sb = consts.tile([P, E], f32)
            nc.gpsimd.dma_start(out=bet_sb, in_=bet.ap().partition_broadcast(P))

        # ---- P3: qk8 = (32Wqk).T @ xqT8 + 32ck ----
        # pass 1 (t-half 0): jp-major over 8 concurrent psums, pipelined
        # with the chunk-pair arrivals; pass 2 (t-half 1): et-major.
        qk_pool = tc.alloc_tile_pool(name="qk", bufs=1, side="left")
        qk8 = qk_pool.tile([P, NE, T], f8)
        pse = [mmp.tile([P, FD], f32, name=f"q0_{et}", tag=f"b{et % 4}")
               for et in range(NE)]
        for jp in range(NP):
            for et in range(NE):
                nc.tensor.matmul(
                    pse[et], wqk_sb[:, 2 * jp:2 * jp + 2, et * P:(et + 1) * P],
                    xqT8[:, 2 * jp:2 * jp + 2, 0:FD],
                    start=(jp == 0), stop=(jp == NP - 1), perf_mode=DR)
                if jp == NP - 1:
                    nc.scalar.activation(qk8[:, et, 0:FD], pse[et],
                                         AF.Identity, bias=ck_sb[:, et:et + 1])
            if jp < NP - 1:
                # zero-junk accumulates (adds 0.0): p-state insurance while
                # the next chunk-pair lands (pair 1 arrives latest)
                for _ in range(8 if jp == 0 else NJF):
                    nc.tensor.matmul(pse[NE - 1], junk8, jmov, start=False,
                                     stop=False, perf_mode=DR)
        for et in range(NE):
            ps2 = mmp.tile([P, FD], f32, name=f"q1_{et}", tag=f"b{et % 4}")
            for jp in range(NP):
                nc.tensor.matmul(
                    ps2, wqk_sb[:, 2 * jp:2 * jp + 2, et * P:(et + 1) * P],
                    xqT8[:, 2 * jp:2 * jp + 2, FD:T],
                    start=(jp == 0), stop=(jp == NP - 1), perf_mode=DR)
            nc.scalar.activation(qk8[:, et, FD:T], ps2,
                                 AF.Identity, bias=ck_sb[:, et:et + 1])

        # ---- P4: scores psum = xkT8.T @ qk8 -> exp(psum/1024 - 2);
        # t-half 0 for all s-tiles first (only needs pass-1 qk8) ----
        expT_pool = tc.alloc_tile_pool(name="expT", bufs=1, side="right")
        expT8 = expT_pool.tile([P, NS, T], f8)
        for tb in range(NBLK_T):
            for st in range(NS):
                ps4 = mmp.tile([P, FD], f32, name=f"s{tb}_{st}",
                               tag=f"b{st % 4}")
                for jp in range(NP):
                    nc.tensor.matmul(
                        ps4, xkT8[:, 2 * jp:2 * jp + 2, st * P:(st + 1) * P],
                        qk8[:, 2 * jp:2 * jp + 2, tb * FD:(tb + 1) * FD],
                        start=(jp == 0), stop=(jp == NP - 1), perf_mode=DR)
                nc.scalar.activation(expT8[:, st, tb * FD:(tb + 1) * FD],
                                     ps4, AF.Exp,
                                     bias=neg2_t, scale=1.0 / 1024.0)

        # ---- RS: rowsum on PE; recip = 1/(16*rowsum) ----
        rwp = tc.alloc_tile_pool(name="rsw", bufs=1, side="right")
        rs_sb = rwp.tile([1, T], f32)
        for tb in range(NBLK_T):
            rps = mmp.tile([P, FD], f32, name=f"rs{tb}", tag=f"b{tb}")
            for stp in range(NSP):
                nc.tensor.matmul(
                    rps[0:1, :], ones8[:, :, 0:1],
                    expT8[:, 2 * stp:2 * stp + 2, tb * FD:(tb + 1) * FD],
                    start=(stp == 0), stop=(stp == NSP - 1), perf_mode=DR)
            # out-proj psum = 16*rowsum*true -> recip of 16*rowsum
            nc.scalar.activation(rs_sb[0:1, tb * FD:(tb + 1) * FD],
                                 rps[0:1, :], AF.Copy, scale=16.0)
        rstp = mmp.tile([P, FD], f32, name="rst", tag="b2")

        # ---- P5+P6 interleaved per T-quarter ----
        ctx_pool = tc.alloc_tile_pool(name="ctxT", bufs=1, side="right")
        ctxT8 = ctx_pool.tile([P, NE, T], f8)
        with (
            tc.tile_pool(name="p6y", bufs=8, side="right") as yp,
            tc.tile_pool(name="p6ln", bufs=8, side="right") as lnp,
            tc.tile_pool(name="p6out", bufs=6, side="right") as outp,
        ):
            ysq = yp.tile([P, FD], bf16, name="ysq", tag="ysq", bufs=1)
            for tb in range(4):
                # P5: ctxT8[:, :, tb quarter] = 0.5 * (xv8.T @ expT8)
                for e in range(NE):
                    ps5w = mmp.tile([P, FD], f32, name=f"c{e}_{tb}",
                                    tag=f"b{e % 4}")
                    ps5 = ps5w[:, 0:QD]
                    for stp in range(NSP):
                        nc.tensor.matmul(
                            ps5, v8[:, 2 * stp:2 * stp + 2, e * P:(e + 1) * P],
                            expT8[:, 2 * stp:2 * stp + 2, tb * QD:(tb + 1) * QD],
                            start=(stp == 0), stop=(stp == NSP - 1), perf_mode=DR)
                    if tb == 0 and e == 0:
                        # PE-transpose rowsum [1,T] -> [P,NT], then recip;
                        # ready long before P6 t0's first STT
                        for j in range(NT):
                            nc.tensor.matmul(rstp[:, j:j + 1],
                                             rs_sb[0:1, j * P:(j + 1) * P],
                                             id1, start=True, stop=True,
                                             is_transpose=True)
                        nc.vector.reciprocal(recip_t, rstp[:, 0:NT])
                    nc.scalar.activation(ctxT8[:, e, tb * QD:(tb + 1) * QD],
                                         ps5, AF.Copy, scale=0.5)
                # P6: the quarter's two t-tiles interleave their matmuls per
                # gc-pair so both tiles' early-quarter y/stats hide under
                # matmuls; STT alternates DVE/GpSimd
                tta, ttb = tb * 2, tb * 2 + 1
                ys, accs = {}, {}
                for tt in (tta, ttb):
                    ys[tt] = yp.tile([P, E], bf16, name=f"y{tt}", tag="y")
                    accs[tt] = lnp.tile([P, 4], f32, name=f"ac{tt}",
                                        tag="ac")
                ps6 = {}
                for tt in (tta, ttb):
                    for h in range(2):
                        ps6[tt, h] = mmp.tile([P, FD], f32, name=f"o{tt}_{h}",
                                              tag=f"b{2 * (tt - tta) + h}")
                        for jp in range(NP):
                            nc.tensor.matmul(
                                ps6[tt, h],
                                ctxT8[:, 2 * jp:2 * jp + 2,
                                      tt * P:(tt + 1) * P],
                                wvo_sb[:, 2 * jp:2 * jp + 2,
                                       h * FD:(h + 1) * FD],
                                start=(jp == 0), stop=(jp == NP - 1),
                                perf_mode=DR)
                for tt in (tta, ttb):
                    y = ys[tt]
                    acc = accs[tt]
                    for h in range(2):
                        # y = psum * (1/(16*rowsum)) + (residual + bo'),
                        # with sum(y) for free via accum_out (DVE only:
                        # GpSimd cannot read PSUM)
                        nc.vector.scalar_tensor_tensor(
                            out=y[:, h * FD:(h + 1) * FD], in0=ps6[tt, h],
                            scalar=recip_t[:, tt:tt + 1],
                            in1=res_t[:, tt, h * FD:(h + 1) * FD],
                            op0=ALU.mult, op1=ALU.add,
                            accum_out=acc[:, h:h + 1])
                        # sum(y^2) on ACT (Square + accum_out) so the LN
                        # stats leave the DVE entirely
                        nc.scalar.activation(
                            ysq, y[:, h * FD:(h + 1) * FD], AF.Square,
                            accum_out=acc[:, 2 + h:3 + h])
                    # var*E = sum(y^2) - sum(y)^2/E, via s2 = sum(y)/32
                    ssum = lnp.tile([P, 1], f32, name=f"ss{tt}", tag="ss")
                    nc.vector.tensor_add(ssum, acc[:, 0:1], acc[:, 1:2])
                    qsum = lnp.tile([P, 1], f32, name=f"qs{tt}", tag="qs")
                    nc.vector.tensor_add(qsum, acc[:, 2:3], acc[:, 3:4])
                    s2 = lnp.tile([P, 1], f32, name=f"s2{tt}", tag="s2")
                    nc.vector.tensor_scalar(out=s2, in0=ssum,
                                            scalar1=1.0 / 32.0, scalar2=None,
                                            op0=ALU.mult)
                    vv = lnp.tile([P, 1], f32, name=f"vv{tt}", tag="vv")
                    nc.vector.scalar_tensor_tensor(
                        out=vv, in0=s2, scalar=s2, in1=qsum,
                        op0=ALU.mult, op1=ALU.subtract)
                    # vv = s2^2 - qsum = -(var*E) -> negative sqrt scale
                    rstd = lnp.tile([P, 1], f32, name=f"rs{tt}", tag="rs")
                    nc.scalar.activation(rstd, vv, AF.Sqrt, bias=eps_t,
                                         scale=-1.0 / 1024.0)
                    nc.vector.reciprocal(rstd, rstd)
                    # o = y*rstd + (-mu*rstd): h0 on ACT || h1 on GpSimd,
                    # stores on sync queue
                    nmr = lnp.tile([P, 1], f32, name=f"nm{tt}", tag="nm")
                    nc.vector.scalar_tensor_tensor(
                        out=nmr, in0=ssum, scalar=-1.0 / 1024.0, in1=rstd,
                        op0=ALU.mult, op1=ALU.mult)
                    o = outp.tile([P, E], f32, name=f"o{tt}", tag="o")
                    nc.scalar.activation(o[:, 0:FD], y[:, 0:FD],
                                         AF.Identity, bias=nmr, scale=rstd)
                    nc.gpsimd.tensor_scalar(
                        out=o[:, FD:E], in0=y[:, FD:E], scalar1=rstd,
                        scalar2=nmr, op0=ALU.mult, op1=ALU.add)
                    if apply_gb:
                        nc.vector.tensor_mul(o, o, gam_sb)
                        nc.vector.tensor_add(o, o, bet_sb)
                    # both halves on the sync queue (a scalar-queue trigger
                    # would block ACT); h1 first — gpsimd finishes it first
                    nc.sync.dma_start(
                        out=out.ap()[tt * P:(tt + 1) * P, FD:E],
                        in_=o[:, FD:E])
                    nc.sync.dma_start(
                        out=out.ap()[tt * P:(tt + 1) * P, 0:FD],
                        in_=o[:, 0:FD])

        ctx_pool.release()
        rwp.release()
        expT_pool.release()
        qk_pool.release()
        v_pool.release()
        res_pool.release()
        xkT_pool.release()
        xqT_pool.release()
        wpool.release()
        mmp.release()
        consts.release()

    nc.compile()
    return nc


def _to_fp8(x):
    return np.clip(x, -240.0, 240.0).astype(ml_dtypes.float8_e4m3)


def kernel(query, key, value, Wq, bq, Wk, bk, Wv, bv, Wo, bo, gamma, beta):
    query = np.asarray(query, dtype=np.float32)
    key = np.asarray(key, dtype=np.float32)
    value = np.asarray(value, dtype=np.float32)
    Wq = np.asarray(Wq, dtype=np.float32)
    bq = np.asarray(bq, dtype=np.float32)
    Wk = np.asarray(Wk, dtype=np.float32)
    Wv = np.asarray(Wv, dtype=np.float32)
    bv = np.asarray(bv, dtype=np.float32)
    Wo = np.asarray(Wo, dtype=np.float32)
    bo = np.asarray(bo, dtype=np.float32)
    gamma = np.asarray(gamma, dtype=np.float32)
    beta = np.asarray(beta, dtype=np.float32)

    # host weight folds (fp64 for exactness)
    Wqk = Wq.T.astype(np.float64) @ Wk.astype(np.float64)        # [e2, e]
    Wvo = Wv.T.astype(np.float64) @ Wo.T.astype(np.float64)      # [e, g]
    ck = Wk.T.astype(np.float64) @ bq.astype(np.float64)         # [e]
    wqk8 = _to_fp8((Wqk * 32.0).astype(np.float32))
    wvo8 = _to_fp8((Wvo * 32.0).astype(np.float32))
    ck2 = np.ascontiguousarray(
        (ck * 32.0).astype(np.float32).reshape(NE, P).T)
    bo2 = (bo + Wo @ bv).astype(np.float32)
    resb = (query + bo2).astype(ml_dtypes.bfloat16)  # residual with bo' folded
    keyT_f8 = np.ascontiguousarray(
        _to_fp8(key).transpose(0, 2, 1))       # [B, E, S] fp8
    val_f8 = _to_fp8(value)
    apply_gb = not (np.all(gamma == 1.0) and np.all(beta == 0.0))

    if apply_gb not in _cache:
        _cache[apply_gb] = _build(apply_gb)
    nc = _cache[apply_gb]

    in_maps = []
    for c in range(8):
        b, h = c // 2, c % 2
        m = {
            "xqT8": np.ascontiguousarray(
                _to_fp8(query[b, h * T:(h + 1) * T]).T),
            "resb": np.ascontiguousarray(resb[b, h * T:(h + 1) * T]),
            "xkT8": keyT_f8[b],
            "xv8": val_f8[b],
            "wqk8": wqk8, "wvo8": wvo8, "ck2": ck2,
        }
        if apply_gb:
            m["gam"] = gamma
            m["bet"] = beta
        in_maps.append(m)

    global _saved_in_maps
    _saved_in_maps = in_maps
    res = run_bass_kernel_spmd(nc, in_maps, core_ids=list(range(8)))
    B = query.shape[0]
    full = np.empty((B, 2 * T, E), dtype=np.float32)
    for c in range(8):
        b, h = c // 2, c % 2
        full[b, h * T:(h + 1) * T] = res.results[c]["out"]
    return full


# revision 13
# speedup vs baseline: 1.0012x; 1.0012x over previous
"""Trainium2 Bass kernel for nn_MultiHeadAttention_5360119185803.

Full-d_model attention (no head split) + residual + LayerNorm, B=4, T=S=2048,
E=1024, fp32 in/out.

Sharding: 8 cores; core c owns batch b=c//2 and query rows
[(c%2)*1024, (c%2+1)*1024). K/V is full per batch; the core pair duplicates
the (tiny) K/V-side work (collectives measured slower than recompute).

v8 design (fp32r 462us -> fp8 DR v5 236us -> v6 122us -> v7 121us -> this).
The PE is at the fp8 DR roofline (~86us of matmul per core); v8 removes the
remaining non-PE time found in the v7 trace:
  * Input DMA is HBM bound (~280 GB/s aggregate, ~140/queue): the critical
    xq+wqk (2MB) lands ~15.5us in, and xk (2MB) behind it gated P4.  v8
    interleaves streams across both queues so each jp chunk-pair (wqk pair +
    xq pair) lands every ~1.8us, and splits xk across both queues right
    after (lands ~21.5us, before P4 needs it).
  * P3 restructured to consume chunk-pairs AS THEY ARRIVE: pass 1 (t-half 0)
    runs jp-major with 8 concurrent psum banks — each jp batch (8 matmuls,
    1.73us) matches the pair arrival cadence, with a couple of zero-junk
    accumulate matmuls (adds 0.0 to open psums) after each batch as p-state
    insurance; pass 2 (t-half 1) runs et-major with everything resident,
    overlapping pass 1's evicts.  P3 ends ~24us vs v7's ~28.5.
  * P4 runs t-half 0 for all 16 s-tiles, then t-half 1, so it only ever
    waits on already-evicted qk8 columns.
  * Tail: the two tiles of each quarter interleave their P6 matmuls per
    gc-pair so both tiles' early-quarter y/stats hide under matmuls; y-STT
    alternates DVE (even quarters) / GpSimd (odd) to halve the exposed DVE
    chain; final normalize h0 on ACT || h1 on GpSimd (tensor_scalar); out
    stores h0 on the sync queue || h1 on the scalar queue.
Carried from v7: single shared PSUM pool (4 tags x bufs=2 = 8 banks, later
phases rotate into earlier tags; rotation deps land on long-done evicts),
PE-transpose of the rowsum (no DRAM roundtrip), residual prefetched in bf16.
Carried from v6 (math): Wqk = Wq.T@Wk, Wvo = Wv.T@Wo.T, bo' = bo + Wo@bv,
bk dropped; all GEMMs fp8e4 DoubleRow (K=256/pass, 1 col/cyc = 157 TF/s);
weights stored as 32*W; exp(psum/1024 - 2) evict; recip = 1/(16*rowsum).

kernel() is self-contained: host prep = shard + dtype converts + weight folds.
"""

import sys

sys.path.insert(0, "/opt/trn_rl_repo")

import ml_dtypes
import numpy as np

import concourse.bacc as bacc
import concourse.bass as bass
import concourse.tile as tile
from concourse import mybir
from concourse.bass_utils import run_bass_kernel_spmd

P = 128
E = 1024          # d_model
S = 2048          # kv seq len per batch
T = 1024          # query rows per core
NE = E // P       # 8 chunks of contraction dim
NT = T // P       # 8 t tiles
NS = S // P       # 16 s tiles
FD = 512          # matmul moving free dim / PSUM bank
QD = 256          # quarter width in t/g columns
NBLK_T = T // FD  # 2 blocks of 512
NP = NE // 2      # 4 DoubleRow pair-chunks over e/f
NSP = NS // 2     # 8 DoubleRow pair-chunks over s
NWUP = 24         # warmup junk matmuls before P3
NJF = 3           # zero-junk accumulates after each jp batch in P3 pass 1

f32 = mybir.dt.float32
bf16 = mybir.dt.bfloat16
f8 = mybir.dt.float8e4
AF = mybir.ActivationFunctionType
ALU = mybir.AluOpType
DR = mybir.MatmulPerfMode.DoubleRow

_cache = {}


def _build(apply_gb):
    nc = bacc.Bacc("TRN2", target_bir_lowering=False, debug=False, num_devices=8)

    xqT8d = nc.dram_tensor("xqT8", [E, T], f8, kind="ExternalInput")
    xkT8d = nc.dram_tensor("xkT8", [E, S], f8, kind="ExternalInput")
    xv8d = nc.dram_tensor("xv8", [S, E], f8, kind="ExternalInput")
    resd = nc.dram_tensor("resb", [T, E], bf16, kind="ExternalInput")  # xq+bo'
    wqk8 = nc.dram_tensor("wqk8", [E, E], f8, kind="ExternalInput")  # 32*Wq.T@Wk
    wvo8 = nc.dram_tensor("wvo8", [E, E], f8, kind="ExternalInput")  # 32*Wv.T@Wo.T
    ck2 = nc.dram_tensor("ck2", [P, NE], f32, kind="ExternalInput")  # 32*Wk.T@bq
    if apply_gb:
        gam = nc.dram_tensor("gam", [E], f32, kind="ExternalInput")
        bet = nc.dram_tensor("bet", [E], f32, kind="ExternalInput")
    out = nc.dram_tensor("out", [T, E], f32, kind="ExternalOutput")

    with tile.TileContext(nc) as tc:
        consts = tc.alloc_tile_pool(name="consts", bufs=1, side="left")
        junk8 = consts.tile([P, 2, P], f8)
        nc.gpsimd.memset(junk8, 0.0)  # first: warmup depends only on this
        jmov = consts.tile([P, 2, FD], f8)
        nc.vector.memset(jmov, 0.0)  # DVE is idle early; needed only ~12us

        # ---- PSUM: one pool, 4 tags x bufs=2 = all 8 banks.  Every phase
        # rotates through tags b0..b3; rotation deps land on long-completed
        # evicts, so no phase-boundary stalls ----
        mmp = tc.alloc_tile_pool(name="mmp", bufs=2, space="PSUM")
        jfill = mmp.tile([P, FD], f32, name="jfill", tag="b0")

        def junk_mm():
            nc.tensor.matmul(jfill[:, 0:P], junk8, junk8, start=True,
                             stop=True, perf_mode=DR)

        # PE warmup burst (HAM ramp) — no input deps
        for _ in range(NWUP):
            junk_mm()

        # ---- input DMA, interleaved across the two queues so each jp
        # chunk-pair (wqk + xq) completes every ~1.8us, then xk split ----
        wpool = tc.alloc_tile_pool(name="wpool", bufs=1, side="left")
        wqk_sb = wpool.tile([P, NE, E], f8)
        wqk_r = wqk8.ap().rearrange("(j p) f -> j p f", p=P)
        xqT_pool = tc.alloc_tile_pool(name="xqT", bufs=1, side="left")
        xqT8 = xqT_pool.tile([P, NE, T], f8)
        xq_r = xqT8d.ap().rearrange("(j p) t -> j p t", p=P)
        xkT_pool = tc.alloc_tile_pool(name="xkT", bufs=1, side="left")
        xkT8 = xkT_pool.tile([P, NE, S], f8)
        xk_r = xkT8d.ap().rearrange("(j p) s -> j p s", p=P)

        # pair k = (wqk c2k,c2k+1 + xq c2k,c2k+1) feeds P3 pass-1's jp=k
        # batch.  Chunks round-robin over sync/scalar/gpsimd so pair k
        # completes ~10.8 + 1.5k us; priority gates (tiny reads of the
        # critical tiles) keep each queue's later streams from starving the
        # in-flight critical slices (HBM arbitration is not fair).
        PAIR_Q = [  # (tensor, chunk, queue) in per-queue trigger order
            (0, "wqk", "sync"), (1, "wqk", "scalar"), (0, "xq", "gpsimd"),
            (1, "xq", "sync"),
            (2, "wqk", "scalar"), (3, "wqk", "gpsimd"), (2, "xq", "sync"),
            (3, "xq", "scalar"),
            (5, "wqk", "gpsimd"), (4, "wqk", "sync"), (4, "xq", "scalar"),
            (5, "xq", "sync"),
            (6, "wqk", "gpsimd"), (7, "wqk", "scalar"), (7, "xq", "gpsimd"),
            (6, "xq", "sync"),
        ]
        for j, which, q in PAIR_Q:
            eng = {"sync": nc.sync, "scalar": nc.scalar,
                   "gpsimd": nc.gpsimd}[q]
            if which == "wqk":
                eng.dma_start(out=wqk_sb[:, j, :], in_=wqk_r[j])
            else:
                eng.dma_start(out=xqT8[:, j, :], in_=xq_r[j])
        ck_sb = consts.tile([P, NE], f32)
        nc.scalar.dma_start(out=ck_sb, in_=ck2.ap())
        # gates: xk waits for all critical chunks; res/xv wait for xk
        gdr = nc.dram_tensor("gate_scratch", [4, NE, 1], f8)
        gts = consts.tile([1, NE, 1], f8)
        nc.sync.dma_start(out=gdr.ap()[0], in_=xqT8[0:1, :, 0:1])
        nc.sync.dma_start(out=gdr.ap()[1], in_=wqk_sb[0:1, :, 0:1])
        nc.gpsimd.tensor_copy(gts, xqT8[0:1, :, 0:1])
        nc.gpsimd.tensor_copy(gts, wqk_sb[0:1, :, 0:1])
        for j in range(4):
            nc.gpsimd.dma_start(out=xkT8[:, j + 4, :], in_=xk_r[j + 4])
            nc.sync.dma_start(out=xkT8[:, j, :], in_=xk_r[j])
        nc.sync.dma_start(out=gdr.ap()[2], in_=xkT8[0:1, :, 0:1])
        nc.gpsimd.tensor_copy(gts, xkT8[0:1, :, 0:1])
        # small consts (needed from P4 on)
        eps_t = consts.tile([P, 1], f32)
        nc.gpsimd.memset(eps_t, 1e-6)
        neg2_t = consts.tile([P, 1], f32)
        nc.gpsimd.memset(neg2_t, -2.0)
        ones8 = consts.tile([P, 2, 16], f8)
        nc.gpsimd.memset(ones8, 1.0)
        id1 = consts.tile([1, 1], f32)
        nc.gpsimd.memset(id1, 1.0)
        recip_t = consts.tile([P, NT], f32)
        # res prefetch (sync) and xv/wvo (gpsimd): needed only from ~52us on
        res_pool = tc.alloc_tile_pool(name="resp", bufs=1, side="right")
        res_t = res_pool.tile([P, NT, E], bf16)
        res_r = resd.ap().rearrange("(tt p) e -> tt p e", p=P)
        for tt in range(NT):
            nc.sync.dma_start(out=res_t[:, tt, :], in_=res_r[tt])
        v_pool = tc.alloc_tile_pool(name="v8", bufs=1, side="left")
        v8 = v_pool.tile([P, NS, E], f8)
        xv_r = xv8d.ap().rearrange("(st p) e -> st p e", p=P)
        for st in range(NS):
            nc.gpsimd.dma_start(out=v8[:, st, :], in_=xv_r[st])
        wvo_sb = wpool.tile([P, NE, E], f8)
        wvo_r = wvo8.ap().rearrange("(j p) f -> j p f", p=P)
        for j in range(NE):
            nc.gpsimd.dma_start(out=wvo_sb[:, j, :], in_=wvo_r[j])
        if apply_gb:
            gam_sb = consts.tile([P, E], f32)
            nc.gpsimd.dma_start(out=gam_sb, in_=gam.ap().partition_broadcast(P))
            bet_sb = consts.tile([P, E], f32)
            nc.gpsimd.dma_start(out=bet_sb, in_=bet.ap().partition_broadcast(P))

        # ---- P3: qk8 = (32Wqk).T @ xqT8 + 32ck ----
        # pass 1 (t-half 0): jp-major over 8 concurrent psums, pipelined
        # with the chunk-pair arrivals; pass 2 (t-half 1): et-major.
        qk_pool = tc.alloc_tile_pool(name="qk", bufs=1, side="left")
        qk8 = qk_pool.tile([P, NE, T], f8)
        pse = [mmp.tile([P, FD], f32, name=f"q0_{et}", tag=f"b{et % 4}")
               for et in range(NE)]
        for jp in range(NP):
            for et in range(NE):
                nc.tensor.matmul(
                    pse[et], wqk_sb[:, 2 * jp:2 * jp + 2, et * P:(et + 1) * P],
                    xqT8[:, 2 * jp:2 * jp + 2, 0:FD],
                    start=(jp == 0), stop=(jp == NP - 1), perf_mode=DR)
                if jp == NP - 1:
                    nc.scalar.activation(qk8[:, et, 0:FD], pse[et],
                                         AF.Identity, bias=ck_sb[:, et:et + 1])
            if jp < NP - 1:
                # zero-junk accumulates (adds 0.0): p-state insurance while
                # the next chunk-pair lands
                for _ in range(NJF):
                    nc.tensor.matmul(pse[NE - 1], junk8, jmov, start=False,
                                     stop=False, perf_mode=DR)
        for et in range(NE):
            ps2 = mmp.tile([P, FD], f32, name=f"q1_{et}", tag=f"b{et % 4}")
            for jp in range(NP):
                nc.tensor.matmul(
                    ps2, wqk_sb[:, 2 * jp:2 * jp + 2, et * P:(et + 1) * P],
                    xqT8[:, 2 * jp:2 * jp + 2, FD:T],
                    start=(jp == 0), stop=(jp == NP - 1), perf_mode=DR)
            nc.scalar.activation(qk8[:, et, FD:T], ps2,
                                 AF.Identity, bias=ck_sb[:, et:et + 1])

        # ---- P4: scores psum = xkT8.T @ qk8 -> exp(psum/1024 - 2);
        # t-half 0 for all s-tiles first (only needs pass-1 qk8) ----
        expT_pool = tc.alloc_tile_pool(name="expT", bufs=1, side="right")
        expT8 = expT_pool.tile([P, NS, T], f8)
        for tb in range(NBLK_T):
            for st in range(NS):
                ps4 = mmp.tile([P, FD], f32, name=f"s{tb}_{st}",
                               tag=f"b{st % 4}")
                for jp in range(NP):
                    nc.tensor.matmul(
                        ps4, xkT8[:, 2 * jp:2 * jp + 2, st * P:(st + 1) * P],
                        qk8[:, 2 * jp:2 * jp + 2, tb * FD:(tb + 1) * FD],
                        start=(jp == 0), stop=(jp == NP - 1), perf_mode=DR)
                nc.scalar.activation(expT8[:, st, tb * FD:(tb + 1) * FD],
                                     ps4, AF.Exp,
                                     bias=neg2_t, scale=1.0 / 1024.0)

        # ---- RS: rowsum on PE; recip = 1/(16*rowsum) ----
        rwp = tc.alloc_tile_pool(name="rsw", bufs=1, side="right")
        rs_sb = rwp.tile([1, T], f32)
        for tb in range(NBLK_T):
            rps = mmp.tile([P, FD], f32, name=f"rs{tb}", tag=f"b{tb}")
            for stp in range(NSP):
                nc.tensor.matmul(
                    rps[0:1, :], ones8[:, :, 0:1],
                    expT8[:, 2 * stp:2 * stp + 2, tb * FD:(tb + 1) * FD],
                    start=(stp == 0), stop=(stp == NSP - 1), perf_mode=DR)
            # out-proj psum = 16*rowsum*true -> recip of 16*rowsum
            nc.scalar.activation(rs_sb[0:1, tb * FD:(tb + 1) * FD],
                                 rps[0:1, :], AF.Copy, scale=16.0)
        rstp = mmp.tile([P, FD], f32, name="rst", tag="b2")

        # ---- P5+P6 interleaved per T-quarter ----
        ctx_pool = tc.alloc_tile_pool(name="ctxT", bufs=1, side="right")
        ctxT8 = ctx_pool.tile([P, NE, T], f8)
        with (
            tc.tile_pool(name="p6y", bufs=8, side="right") as yp,
            tc.tile_pool(name="p6ln", bufs=8, side="right") as lnp,
            tc.tile_pool(name="p6out", bufs=6, side="right") as outp,
        ):
            ysq = yp.tile([P, FD], bf16, name="ysq", tag="ysq", bufs=1)
            for tb in range(4):
                # P5: ctxT8[:, :, tb quarter] = 0.5 * (xv8.T @ expT8)
                for e in range(NE):
                    ps5w = mmp.tile([P, FD], f32, name=f"c{e}_{tb}",
                                    tag=f"b{e % 4}")
                    ps5 = ps5w[:, 0:QD]
                    for stp in range(NSP):
                        nc.tensor.matmul(
                            ps5, v8[:, 2 * stp:2 * stp + 2, e * P:(e + 1) * P],
                            expT8[:, 2 * stp:2 * stp + 2, tb * QD:(tb + 1) * QD],
                            start=(stp == 0), stop=(stp == NSP - 1), perf_mode=DR)
                    if tb == 0 and e == 0:
                        # PE-transpose rowsum [1,T] -> [P,NT], then recip;
                        # ready long before P6 t0's first STT
                        for j in range(NT):
                            nc.tensor.matmul(rstp[:, j:j + 1],
                                             rs_sb[0:1, j * P:(j + 1) * P],
                                             id1, start=True, stop=True,
                                             is_transpose=True)
                        nc.vector.reciprocal(recip_t, rstp[:, 0:NT])
                    nc.scalar.activation(ctxT8[:, e, tb * QD:(tb + 1) * QD],
                                         ps5, AF.Copy, scale=0.5)
                # P6: the quarter's two t-tiles interleave their matmuls per
                # gc-pair so both tiles' early-quarter y/stats hide under
                # matmuls; STT alternates DVE/GpSimd
                tta, ttb = tb * 2, tb * 2 + 1
                ys, accs = {}, {}
                for tt in (tta, ttb):
                    ys[tt] = yp.tile([P, E], bf16, name=f"y{tt}", tag="y")
                    accs[tt] = lnp.tile([P, 4], f32, name=f"ac{tt}",
                                        tag="ac")
                ps6 = {}
                for tt in (tta, ttb):
                    for h in range(2):
                        ps6[tt, h] = mmp.tile([P, FD], f32, name=f"o{tt}_{h}",
                                              tag=f"b{2 * (tt - tta) + h}")
                        for jp in range(NP):
                            nc.tensor.matmul(
                                ps6[tt, h],
                                ctxT8[:, 2 * jp:2 * jp + 2,
                                      tt * P:(tt + 1) * P],
                                wvo_sb[:, 2 * jp:2 * jp + 2,
                                       h * FD:(h + 1) * FD],
                                start=(jp == 0), stop=(jp == NP - 1),
                                perf_mode=DR)
                for tt in (tta, ttb):
                    y = ys[tt]
                    acc = accs[tt]
                    for h in range(2):
                        # y = psum * (1/(16*rowsum)) + (residual + bo'),
                        # with sum(y) for free via accum_out (DVE only:
                        # GpSimd cannot read PSUM)
                        nc.vector.scalar_tensor_tensor(
                            out=y[:, h * FD:(h + 1) * FD], in0=ps6[tt, h],
                            scalar=recip_t[:, tt:tt + 1],
                            in1=res_t[:, tt, h * FD:(h + 1) * FD],
                            op0=ALU.mult, op1=ALU.add,
                            accum_out=acc[:, h:h + 1])
                        # sum(y^2) on ACT (Square + accum_out) so the LN
                        # stats leave the DVE entirely
                        nc.scalar.activation(
                            ysq, y[:, h * FD:(h + 1) * FD], AF.Square,
                            accum_out=acc[:, 2 + h:3 + h])
                    # var*E = sum(y^2) - sum(y)^2/E, via s2 = sum(y)/32
                    ssum = lnp.tile([P, 1], f32, name=f"ss{tt}", tag="ss")
                    nc.vector.tensor_add(ssum, acc[:, 0:1], acc[:, 1:2])
                    qsum = lnp.tile([P, 1], f32, name=f"qs{tt}", tag="qs")
                    nc.vector.tensor_add(qsum, acc[:, 2:3], acc[:, 3:4])
                    s2 = lnp.tile([P, 1], f32, name=f"s2{tt}", tag="s2")
                    nc.vector.tensor_scalar(out=s2, in0=ssum,
                                            scalar1=1.0 / 32.0, scalar2=None,
                                            op0=ALU.mult)
                    vv = lnp.tile([P, 1], f32, name=f"vv{tt}", tag="vv")
                    nc.vector.scalar_tensor_tensor(
                        out=vv, in0=s2, scalar=s2, in1=qsum,
                        op0=ALU.mult, op1=ALU.subtract)
                    # vv = s2^2 - qsum = -(var*E) -> negative sqrt scale
                    rstd = lnp.tile([P, 1], f32, name=f"rs{tt}", tag="rs")
                    nc.scalar.activation(rstd, vv, AF.Sqrt, bias=eps_t,
                                         scale=-1.0 / 1024.0)
                    nc.vector.reciprocal(rstd, rstd)
                    # o = y*rstd + (-mu*rstd): h0 on ACT || h1 on GpSimd,
                    # stores on sync queue
                    nmr = lnp.tile([P, 1], f32, name=f"nm{tt}", tag="nm")
                    nc.vector.scalar_tensor_tensor(
                        out=nmr, in0=ssum, scalar=-1.0 / 1024.0, in1=rstd,
                        op0=ALU.mult, op1=ALU.mult)
                    o = outp.tile([P, E], f32, name=f"o{tt}", tag="o")
                    nc.scalar.activation(o[:, 0:FD], y[:, 0:FD],
                                         AF.Identity, bias=nmr, scale=rstd)
                    nc.gpsimd.tensor_scalar(
                        out=o[:, FD:E], in0=y[:, FD:E], scalar1=rstd,
                        scalar2=nmr, op0=ALU.mult, op1=ALU.add)
                    if apply_gb:
                        nc.vector.tensor_mul(o, o, gam_sb)
                        nc.vector.tensor_add(o, o, bet_sb)
                    # both halves on the sync queue (a scalar-queue trigger
                    # would block ACT); h1 first — gpsimd finishes it first
                    nc.sync.dma_start(
                        out=out.ap()[tt * P:(tt + 1) * P, FD:E],
                        in_=o[:, FD:E])
                    nc.sync.dma_start(
                        out=out.ap()[tt * P:(tt + 1) * P, 0:FD],
                        in_=o[:, 0:FD])

        ctx_pool.release()
        rwp.release()
        expT_pool.release()
        qk_pool.release()
        v_pool.release()
        res_pool.release()
        xkT_pool.release()
        xqT_pool.release()
        wpool.release()
        mmp.release()
        consts.release()

    nc.compile()
    return nc


def _to_fp8(x):
    return np.clip(x, -240.0, 240.0).astype(ml_dtypes.float8_e4m3)


def kernel(query, key, value, Wq, bq, Wk, bk, Wv, bv, Wo, bo, gamma, beta):
    query = np.asarray(query, dtype=np.float32)
    key = np.asarray(key, dtype=np.float32)
    value = np.asarray(value, dtype=np.float32)
    Wq = np.asarray(Wq, dtype=np.float32)
    bq = np.asarray(bq, dtype=np.float32)
    Wk = np.asarray(Wk, dtype=np.float32)
    Wv = np.asarray(Wv, dtype=np.float32)
    bv = np.asarray(bv, dtype=np.float32)
    Wo = np.asarray(Wo, dtype=np.float32)
    bo = np.asarray(bo, dtype=np.float32)
    gamma = np.asarray(gamma, dtype=np.float32)
    beta = np.asarray(beta, dtype=np.float32)

    # host weight folds (fp64 for exactness)
    Wqk = Wq.T.astype(np.float64) @ Wk.astype(np.float64)        # [e2, e]
    Wvo = Wv.T.astype(np.float64) @ Wo.T.astype(np.float64)      # [e, g]
    ck = Wk.T.astype(np.float64) @ bq.astype(np.float64)         # [e]
    wqk8 = _to_fp8((Wqk * 32.0).astype(np.float32))
    wvo8 = _to_fp8((Wvo * 32.0).astype(np.float32))
    ck2 = np.ascontiguousarray(
        (ck * 32.0).astype(np.float32).reshape(NE, P).T)
    bo2 = (bo + Wo @ bv).astype(np.float32)
    resb = (query + bo2).astype(ml_dtypes.bfloat16)  # residual with bo' folded
    keyT_f8 = np.ascontiguousarray(
        _to_fp8(key).transpose(0, 2, 1))       # [B, E, S] fp8
    val_f8 = _to_fp8(value)
    apply_gb = not (np.all(gamma == 1.0) and np.all(beta == 0.0))

    if apply_gb not in _cache:
        _cache[apply_gb] = _build(apply_gb)
    nc = _cache[apply_gb]

    in_maps = []
    for c in range(8):
        b, h = c // 2, c % 2
        m = {
            "xqT8": np.ascontiguousarray(
                _to_fp8(query[b, h * T:(h + 1) * T]).T),
            "resb": np.ascontiguousarray(resb[b, h * T:(h + 1) * T]),
            "xkT8": keyT_f8[b],
            "xv8": val_f8[b],
            "wqk8": wqk8, "wvo8": wvo8, "ck2": ck2,
        }
        if apply_gb:
            m["gam"] = gamma
            m["bet"] = beta
        in_maps.append(m)

    global _saved_in_maps
    _saved_in_maps = in_maps
    res = run_bass_kernel_spmd(nc, in_maps, core_ids=list(range(8)))
    B = query.shape[0]
    full = np.empty((B, 2 * T, E), dtype=np.float32)
    for c in range(8):
        b, h = c // 2, c % 2
        full[b, h * T:(h + 1) * T] = res.results[c]["out"]
    return full
